# revision 6
# baseline (speedup 1.0000x reference)
"""CTRNN forward kernel for Trainium2 (8 NeuronCores, data-parallel over batch).

Reference computation (per step t, dt=0.02):
    h = h*(1-dt) + dt*(tanh(h) @ J.T + v_t @ Bmat.T)
    out_t = tanh(h) @ W_ro.T

Device mapping (per core, B_LOC=16 of the 128 batch rows):
  - Layout: everything lives as [hidden-on-partitions x batch-on-free] ("hT").
    hidden 512 = 4 chunks of 128 partitions; chunk q row p = hidden index 128*q+p.
  - PSUM holds h (transposed) in 4 quarter banks, one per hidden chunk
    ([128, 16] each). It is never evicted:
      per step, per quarter q:
        psum_q += tanh_prev @ (dt*J/0.98).T rows   (PE matmuls, accumulate)
        psum_q = 0.98*psum_q + dt*Bmat_q x v_t     (DVE scalar_tensor_tensor)
        y_q = tanh(psum_q)                         (ACT, psum -> SBUF)
        psum_ro[:, t] += y_q.T @ W_ro_q.T          (PE, small)
    Quarter granularity lets each chunk's DVE/ACT overlap the other
    chunks' matmuls, and step t+1's PE work (k-chunks 0..2) can start
    before step t's tanh of chunk 3 completes.
  - J matmuls default to f32r (FP22 single-pass; fp32 is 4 passes with a
    double stationary load). Validated: rel err ~6e-4 « 2e-2 gate.
  - The host wrapper pre-transposes/pre-scales the small weights once:
    JT = (dt*J/0.98).T, bmr = dt*Bmat row, wrt = W_ro in [128 x 4] hT
    layout, vel transposed to [T, B_LOC].
  - bv outer products dt*Bmat x v_t are precomputed per LBV-step block with
    PE outer products + DVE copies.
"""

import math
import os
import sys

import numpy as np

sys.path.insert(0, "/opt/trn_rl_repo")

DT = 0.02
DECAY = 1.0 - DT          # 0.98
HIDDEN = 512
BATCH = 128
T_FULL = 1024
N_CORES = 8
B_LOC = BATCH // N_CORES  # 16
CB = HIDDEN // 128        # 4 hidden chunks


def build_nc(T=T_FULL, m_tile=128, lbv=128, ro_bank=512, mm_dtype="f32r"):
    import concourse.bass as bass
    import concourse.tile as tile
    from concourse import bacc, mybir

    f32 = mybir.dt.float32
    # f32r is a zero-copy bitcast (same 4-byte layout; single-pass FP22 PE
    # matmul instead of fp32's 4-pass H/L decomposition). ISA restriction:
    # f32r matmul dst start_partition must be 0 -> full 128-row out tiles
    # (no tile_position col-packing), and dst free count must be even ->
    # readout/bv/init matmuls stay fp32.
    mm_dt = {"f32": mybir.dt.float32, "f32r": mybir.dt.float32r}[mm_dtype]
    if mm_dtype == "f32r":
        assert m_tile == 128, "f32r J-matmuls require dst start_partition 0"
    nc = bacc.Bacc()

    # Host-prepped inputs (see make_in_maps): JT=(dt*J/0.98).T, bmr=dt*Bmat
    # [1,512], wrt=W_ro [128,4], velT=[T, B_LOC].
    jt_h = nc.declare_dram_parameter("JT", [HIDDEN, HIDDEN], mm_dt, isOutput=False)
    bmr_h = nc.declare_dram_parameter("bmr", [1, HIDDEN], f32, isOutput=False)
    wrt_h = nc.declare_dram_parameter("wrt", [128, CB], f32, isOutput=False)
    velt_h = nc.declare_dram_parameter("velT", [T, B_LOC], f32, isOutput=False)
    out_h = nc.declare_dram_parameter("out", [B_LOC, T], f32, isOutput=True)

    n_ro = (T + ro_bank - 1) // ro_bank

    with tile.TileContext(nc) as tc:
        with (
            tc.tile_pool(name="singles", bufs=1) as singles,
            tc.tile_pool(name="yp", bufs=2) as yp,
            tc.tile_pool(name="velp", bufs=2) as velp,
            tc.tile_pool(name="bvpp", bufs=2) as bvpp,
            tc.tile_pool(name="psum", bufs=1, space="PSUM") as pp,
        ):
            # ---- weights staging ----
            jt = singles.tile([128, CB, HIDDEN], mm_dt, tag="jt")  # (dt*J/.98)^T
            nc.sync.dma_start(
                out=jt, in_=jt_h.rearrange("(c p) i -> p c i", p=128)
            )
            bmr = singles.tile([1, HIDDEN], f32, tag="bmr")  # dt*Bmat as a row
            nc.sync.dma_start(out=bmr, in_=bmr_h[:, :])
            wrt = singles.tile([128, CB], f32, tag="wrt")
            nc.sync.dma_start(out=wrt, in_=wrt_h[:, :])

            # zero lhsT/rhs used to clear+claim the h PSUM banks (start=True)
            zrow = singles.tile([1, 128], f32, tag="zrow")
            nc.vector.memset(zrow, 0.0)

            psum_z = [
                pp.tile([128, B_LOC], f32, tag=f"z{q}", name=f"psum_z{q}")
                for q in range(CB)
            ]
            psum_ro = [
                pp.tile([B_LOC, ro_bank], f32, tag=f"ro{i}", name=f"psum_ro{i}")
                for i in range(n_ro)
            ]
            # junk PSUM target for "absorber" matmuls: each absorber consumes a
            # single fresh semaphore tick (DMA completion etc.) so that real
            # matmuls never need more than ONE sync wait (the self-loading
            # matmul's LDWEIGHTS slice has a single wait slot).
            pjunk = pp.tile([1, 8], f32, tag="junk", name="psum_junk")

            def absorb(src_1el):
                if src_1el.dtype != f32:
                    src_1el = src_1el.bitcast(f32)
                nc.tensor.matmul(
                    out=pjunk[0:1, 0:1],
                    lhsT=src_1el,
                    rhs=src_1el,
                    start=True,
                    stop=True,
                    skip_group_check=True,
                )

            for q in range(CB):
                nc.tensor.matmul(
                    out=psum_z[q],
                    lhsT=zrow[0:1, 0:128],
                    rhs=zrow[0:1, 0:B_LOC],
                    start=True,
                    stop=True,
                    skip_group_check=True,
                )

            # soak up the weight-staging DMA completions one at a time
            absorb(jt[0:1, 0, 0:1])
            absorb(wrt[0:1, 0:1])
            absorb(bmr[0:1, 0:1])

            y_prev = yp.tile([128, CB, B_LOC], mm_dt, tag="y")  # tanh(h_{t-1})
            nc.vector.memset(y_prev.rearrange("p c b -> p (c b)").bitcast(f32), 0.0)

            velb = None
            bvp = None
            bvp_prev = None
            for t in range(T):
                j = t % lbv
                if j == 0:
                    # stage next LBV steps of dt*Bmat x v outer products, built
                    # on the PE: bvp[p, t, c, b] = bmr[128c+p] * v[t, b]
                    velb = velp.tile([1, lbv * B_LOC], f32, tag="velB")
                    nc.sync.dma_start(
                        out=velb,
                        in_=velt_h[t : t + lbv, :]
                        .rearrange("t b -> (t b)")
                        .unsqueeze(0),
                    )
                    if bvp_prev is not None:
                        # observe the previous block's last bvp copy (DVE tick)
                        # so the matmuls below only wait on the velb DMA
                        absorb(bvp_prev[0:1, lbv - 1, CB - 1, 0:1])
                    bvp_prev = bvp
                    bvp = bvpp.tile([128, lbv, CB, B_LOC], f32, tag="bvp")
                    for c in range(CB):
                        for q0 in range(0, lbv * B_LOC, 512):
                            qn = min(512, lbv * B_LOC - q0)
                            t0, nt = q0 // B_LOC, qn // B_LOC
                            pbv = pp.tile(
                                [128, 512], f32, tag="pbv", bufs=1, name="psum_bv"
                            )
                            nc.tensor.matmul(
                                out=pbv[:, 0:qn],
                                lhsT=bmr[0:1, 128 * c : 128 * (c + 1)],
                                rhs=velb[0:1, q0 : q0 + qn],
                                start=True,
                                stop=True,
                                skip_group_check=True,
                            )
                            nc.vector.tensor_copy(
                                bvp[:, t0 : t0 + nt, c, :],
                                pbv[:, 0:qn].rearrange("p (t b) -> p t b", b=B_LOC),
                            )

                y_new = yp.tile([128, CB, B_LOC], mm_dt, tag="y")
                rb, rc = t // ro_bank, t % ro_bank

                # per quarter (out-chunk q): matmuls accumulate (J pre-divided
                # by 0.98), then decay+input (DVE), then tanh (ACT), then the
                # readout matmul for this quarter.
                for q in range(CB):
                    # psum_q += tanh_prev @ (dt*J/0.98).T  rows 128q..128q+127
                    for c in range(CB):
                        for i0 in range(0, 128, m_tile):
                            nc.tensor.matmul(
                                out=psum_z[q][i0 : i0 + m_tile, :],
                                lhsT=jt[:, c, 128 * q + i0 : 128 * q + i0 + m_tile],
                                rhs=y_prev[:, c, :],
                                start=False,
                                stop=False,
                                skip_group_check=True,
                                tile_position=(0, i0) if m_tile <= 64 else None,
                            )
                    # psum_q = 0.98*psum_q + dt*Bmat_q x v_t
                    nc.vector.scalar_tensor_tensor(
                        out=psum_z[q],
                        in0=psum_z[q],
                        scalar=float(DECAY),
                        in1=bvp[:, j, q, :],
                        op0=mybir.AluOpType.mult,
                        op1=mybir.AluOpType.add,
                    )
                    # y_q = tanh(h_q)
                    nc.scalar.activation(
                        out=y_new[:, q, :],
                        in_=psum_z[q],
                        func=mybir.ActivationFunctionType.Tanh,
                    )
                    # readout partial: psum_ro[:, t] += y_q.T @ W_ro_q.T
                    # (fp32 matmul: f32r disallows odd/unaligned dst free APs)
                    ro_lhsT = y_new[:, q, :]
                    if mm_dt != f32:
                        ro_lhsT = ro_lhsT.bitcast(f32)
                    nc.tensor.matmul(
                        out=psum_ro[rb][0:B_LOC, rc : rc + 1],
                        lhsT=ro_lhsT,
                        rhs=wrt[:, q : q + 1],
                        start=(q == 0),
                        stop=(q == CB - 1),
                        skip_group_check=True,
                    )

                if rc == ro_bank - 1 or t == T - 1:
                    out_sb = velp.tile([B_LOC, ro_bank], f32, tag="osb", name="out_sb")
                    nc.vector.tensor_copy(out_sb[:, 0 : rc + 1], psum_ro[rb][:, 0 : rc + 1])
                    nc.sync.dma_start(
                        out=out_h[:, rb * ro_bank : rb * ro_bank + rc + 1],
                        in_=out_sb[:, 0 : rc + 1],
                    )

                y_prev = y_new

    nc.compile()
    return nc


_NC_CACHE = {}


def _get_nc(**kw):
    key = tuple(sorted(kw.items()))
    if key not in _NC_CACHE:
        _NC_CACHE[key] = build_nc(**kw)
    return _NC_CACHE[key]


def make_in_maps(vel, J, Bmat, W_ro):
    vel = np.asarray(vel, dtype=np.float32)[:, :, 0]          # [B, T]
    J = np.asarray(J, dtype=np.float32)
    Bmat = np.asarray(Bmat, dtype=np.float32)
    W_ro = np.asarray(W_ro, dtype=np.float32)

    jt = np.ascontiguousarray((DT / DECAY * J).T)              # [512, 512], pre-divided by 0.98
    bmr = np.ascontiguousarray((DT * Bmat[:, 0]).reshape(1, HIDDEN))  # [1, 512]
    wrt = np.ascontiguousarray(W_ro[0].reshape(CB, 128).T)     # [128, 4]
    return [
        {
            "JT": jt,
            "bmr": bmr,
            "wrt": wrt,
            "velT": np.ascontiguousarray(vel[c * B_LOC : (c + 1) * B_LOC].T),
        }
        for c in range(N_CORES)
    ]


def kernel(vel, J, Bmat, W_ro, _trace=False, **build_kw):
    from concourse.bass_utils import run_bass_kernel_spmd

    nc = _get_nc(**build_kw)
    in_maps = make_in_maps(vel, J, Bmat, W_ro)
    res = run_bass_kernel_spmd(
        nc, in_maps, list(range(N_CORES)), trace=_trace
    )
    out = np.concatenate([r["out"] for r in res.results], axis=0)
    out = out[:, :, None].astype(np.float32)
    if _trace:
        kernel.last_results = res
    return out


kernel.last_results = None


# revision 7
# speedup vs baseline: 1.8701x; 1.8701x over previous
"""CTRNN forward kernel for Trainium2 (8 NeuronCores, data-parallel over batch).

Reference computation (per step t, dt=0.02):
    h = h*(1-dt) + dt*(tanh(h) @ J.T + v_t @ Bmat.T)
    out_t = tanh(h) @ W_ro.T

Device mapping (per core, B_LOC=16 of the 128 batch rows):
  - Layout: [hidden-on-partitions x batch-on-free] ("hT"); hidden 512 =
    4 chunks of 128 partitions; chunk q row p = hidden index 128*q+p.
  - PSUM holds h (transposed) in 4 quarter banks [128, 16], never evicted.
  - Split-precision bf16 matmuls (weight-load bound kernel: fp32 LDWEIGHTS
    is 2-pass and f32r is 4x slow, bf16 gets FWL). J' = (dt/0.98)*J.T is
    split J' = A + B with A=bf16(J'), B=bf16(J'-A); y = tanh(h) is split
    y = ya + yb likewise. Accumulate A@ya + A@yb + B@ya (B@yb term ~2^-18
    dropped). Measured end-to-end rel err vs fp32 reference: 2.8e-5.
  - Per step, per quarter q:
      psum_q += A_qc@ya_c + A_qc@yb_c + B_qc@ya_c   (12 bf16 matmuls)
      psum_q = 0.98*psum_q + dt*Bmat_q x v_t        (DVE stt)
      y32 = tanh(psum_q)                            (ACT, psum -> SBUF f32)
      ya_q = bf16(y32); yb_q = bf16(y32 - ya_q)     (2 DVE ops)
      psum_ro[:, t] += ya_q.T @ W_ro_q.T            (bf16 readout matmul)
    Quarter granularity staggers DVE/ACT under other quarters' matmuls and
    lets step t+1's PE work start before step t fully finishes.
  - bv outer products dt*Bmat x v_t are precomputed per LBV-step block
    (PE outer product fp32 + DVE copies).
"""

import math
import os
import sys

import numpy as np

sys.path.insert(0, "/opt/trn_rl_repo")

DT = 0.02
DECAY = 1.0 - DT          # 0.98
HIDDEN = 512
BATCH = 128
T_FULL = 1024
N_CORES = 8
B_LOC = BATCH // N_CORES  # 16
CB = HIDDEN // 128        # 4 hidden chunks


def build_nc(T=T_FULL, lbv=128, ro_bank=512):
    import concourse.bass as bass
    import concourse.tile as tile
    from concourse import bacc, mybir

    f32 = mybir.dt.float32
    bf16 = mybir.dt.bfloat16
    nc = bacc.Bacc()

    # Host-prepped inputs (see make_in_maps): JA/JB = bf16 split of
    # (dt*J/0.98).T, bmr = dt*Bmat row, wrt = bf16 W_ro in [128 x 4] hT
    # layout, velT = [T, B_LOC].
    ja_h = nc.declare_dram_parameter("JA", [HIDDEN, HIDDEN], bf16, isOutput=False)
    jb_h = nc.declare_dram_parameter("JB", [HIDDEN, HIDDEN], bf16, isOutput=False)
    bmr_h = nc.declare_dram_parameter("bmr", [1, HIDDEN], f32, isOutput=False)
    wrt_h = nc.declare_dram_parameter("wrt", [128, CB], bf16, isOutput=False)
    velt_h = nc.declare_dram_parameter("velT", [T, B_LOC], f32, isOutput=False)
    out_h = nc.declare_dram_parameter("out", [B_LOC, T], f32, isOutput=True)

    n_ro = (T + ro_bank - 1) // ro_bank

    with tile.TileContext(nc) as tc:
        with (
            tc.tile_pool(name="singles", bufs=1) as singles,
            tc.tile_pool(name="yp", bufs=2) as yp,
            tc.tile_pool(name="y32p", bufs=4) as y32p,
            tc.tile_pool(name="velp", bufs=2) as velp,
            tc.tile_pool(name="bvpp", bufs=2) as bvpp,
            tc.tile_pool(name="psum", bufs=1, space="PSUM") as pp,
        ):
            # ---- weights staging ----
            jta = singles.tile([128, CB, HIDDEN], bf16, tag="jta")
            nc.sync.dma_start(out=jta, in_=ja_h.rearrange("(c p) i -> p c i", p=128))
            jtb = singles.tile([128, CB, HIDDEN], bf16, tag="jtb")
            nc.sync.dma_start(out=jtb, in_=jb_h.rearrange("(c p) i -> p c i", p=128))
            bmr = singles.tile([1, HIDDEN], f32, tag="bmr")  # dt*Bmat as a row
            nc.sync.dma_start(out=bmr, in_=bmr_h[:, :])
            wrt = singles.tile([128, CB], bf16, tag="wrt")
            nc.sync.dma_start(out=wrt, in_=wrt_h[:, :])

            # zero lhsT/rhs used to clear+claim the h PSUM banks (start=True)
            zrow = singles.tile([1, 128], f32, tag="zrow")
            nc.vector.memset(zrow, 0.0)

            psum_z = [
                pp.tile([128, B_LOC], f32, tag=f"z{q}", name=f"psum_z{q}")
                for q in range(CB)
            ]
            psum_ro = [
                pp.tile([B_LOC, ro_bank], f32, tag=f"ro{i}", name=f"psum_ro{i}")
                for i in range(n_ro)
            ]
            # junk PSUM target for "absorber" matmuls: each absorber consumes a
            # single fresh semaphore tick (DMA completion etc.) so that real
            # matmuls never need more than ONE sync wait (the self-loading
            # matmul's LDWEIGHTS slice has a single wait slot).
            pjunk = pp.tile([1, 8], f32, tag="junk", name="psum_junk")

            def absorb(src_1el):
                nc.tensor.matmul(
                    out=pjunk[0:1, 0:1],
                    lhsT=src_1el,
                    rhs=src_1el,
                    start=True,
                    stop=True,
                    skip_group_check=True,
                )

            for q in range(CB):
                nc.tensor.matmul(
                    out=psum_z[q],
                    lhsT=zrow[0:1, 0:128],
                    rhs=zrow[0:1, 0:B_LOC],
                    start=True,
                    stop=True,
                    skip_group_check=True,
                )

            # soak up the weight-staging DMA completions one at a time
            absorb(jta[0:1, 0, 0:1])
            absorb(jtb[0:1, 0, 0:1])
            absorb(wrt[0:1, 0:1])
            absorb(bmr[0:1, 0:1])

            ya_prev = yp.tile([128, CB, B_LOC], bf16, tag="ya")
            yb_prev = yp.tile([128, CB, B_LOC], bf16, tag="yb")
            nc.vector.memset(ya_prev.rearrange("p c b -> p (c b)"), 0.0)
            nc.vector.memset(yb_prev.rearrange("p c b -> p (c b)"), 0.0)

            velb = None
            bvp = None
            bvp_prev = None
            for t in range(T):
                j = t % lbv
                if j == 0:
                    # stage next LBV steps of dt*Bmat x v outer products, built
                    # on the PE: bvp[p, t, c, b] = bmr[128c+p] * v[t, b]
                    velb = velp.tile([1, lbv * B_LOC], f32, tag="velB")
                    nc.sync.dma_start(
                        out=velb,
                        in_=velt_h[t : t + lbv, :]
                        .rearrange("t b -> (t b)")
                        .unsqueeze(0),
                    )
                    if bvp_prev is not None:
                        # observe the previous block's last bvp copy (DVE tick)
                        # so the matmuls below only wait on the velb DMA
                        absorb(bvp_prev[0:1, lbv - 1, CB - 1, 0:1])
                    bvp_prev = bvp
                    bvp = bvpp.tile([128, lbv, CB, B_LOC], f32, tag="bvp")
                    for c in range(CB):
                        for q0 in range(0, lbv * B_LOC, 512):
                            qn = min(512, lbv * B_LOC - q0)
                            t0, nt = q0 // B_LOC, qn // B_LOC
                            pbv = pp.tile(
                                [128, 512], f32, tag="pbv", bufs=1, name="psum_bv"
                            )
                            nc.tensor.matmul(
                                out=pbv[:, 0:qn],
                                lhsT=bmr[0:1, 128 * c : 128 * (c + 1)],
                                rhs=velb[0:1, q0 : q0 + qn],
                                start=True,
                                stop=True,
                                skip_group_check=True,
                            )
                            nc.vector.tensor_copy(
                                bvp[:, t0 : t0 + nt, c, :],
                                pbv[:, 0:qn].rearrange("p (t b) -> p t b", b=B_LOC),
                            )

                ya_new = yp.tile([128, CB, B_LOC], bf16, tag="ya")
                yb_new = yp.tile([128, CB, B_LOC], bf16, tag="yb")
                rb, rc = t // ro_bank, t % ro_bank

                # per quarter (out-chunk q): 12 bf16 matmuls accumulate, then
                # decay+input (DVE), tanh (ACT), ya/yb split (DVE), readout.
                for q in range(CB):
                    for c in range(CB):
                        # A_qc @ ya_c and A_qc @ yb_c adjacent: same stationary
                        lhsA = jta[:, c, 128 * q : 128 * (q + 1)]
                        nc.tensor.matmul(
                            out=psum_z[q],
                            lhsT=lhsA,
                            rhs=ya_prev[:, c, :],
                            start=False,
                            stop=False,
                            skip_group_check=True,
                        )
                        nc.tensor.matmul(
                            out=psum_z[q],
                            lhsT=lhsA,
                            rhs=yb_prev[:, c, :],
                            start=False,
                            stop=False,
                            skip_group_check=True,
                        )
                    for c in range(CB):
                        nc.tensor.matmul(
                            out=psum_z[q],
                            lhsT=jtb[:, c, 128 * q : 128 * (q + 1)],
                            rhs=ya_prev[:, c, :],
                            start=False,
                            stop=False,
                            skip_group_check=True,
                        )
                    # psum_q = 0.98*psum_q + dt*Bmat_q x v_t
                    nc.vector.scalar_tensor_tensor(
                        out=psum_z[q],
                        in0=psum_z[q],
                        scalar=float(DECAY),
                        in1=bvp[:, j, q, :],
                        op0=mybir.AluOpType.mult,
                        op1=mybir.AluOpType.add,
                    )
                    # y32 = tanh(h_q); split into bf16 ya + yb
                    y32 = y32p.tile([128, B_LOC], f32, tag="y32")
                    nc.scalar.activation(
                        out=y32,
                        in_=psum_z[q],
                        func=mybir.ActivationFunctionType.Tanh,
                    )
                    nc.vector.tensor_copy(ya_new[:, q, :], y32)
                    nc.vector.tensor_tensor(
                        out=yb_new[:, q, :],
                        in0=y32,
                        in1=ya_new[:, q, :],
                        op=mybir.AluOpType.subtract,
                    )
                    # readout partial: psum_ro[:, t] += ya_q.T @ W_ro_q.T
                    nc.tensor.matmul(
                        out=psum_ro[rb][0:B_LOC, rc : rc + 1],
                        lhsT=ya_new[:, q, :],
                        rhs=wrt[:, q : q + 1],
                        start=(q == 0),
                        stop=(q == CB - 1),
                        skip_group_check=True,
                    )

                if rc == ro_bank - 1 or t == T - 1:
                    out_sb = velp.tile([B_LOC, ro_bank], f32, tag="osb", name="out_sb")
                    nc.vector.tensor_copy(out_sb[:, 0 : rc + 1], psum_ro[rb][:, 0 : rc + 1])
                    nc.sync.dma_start(
                        out=out_h[:, rb * ro_bank : rb * ro_bank + rc + 1],
                        in_=out_sb[:, 0 : rc + 1],
                    )

                ya_prev = ya_new
                yb_prev = yb_new

    nc.compile()
    return nc


_NC_CACHE = {}


def _get_nc(**kw):
    key = tuple(sorted(kw.items()))
    if key not in _NC_CACHE:
        _NC_CACHE[key] = build_nc(**kw)
    return _NC_CACHE[key]


def _to_bf16(x):
    import ml_dtypes

    return np.asarray(x, dtype=np.float32).astype(ml_dtypes.bfloat16)


def make_in_maps(vel, J, Bmat, W_ro):
    vel = np.asarray(vel, dtype=np.float32)[:, :, 0]          # [B, T]
    J = np.asarray(J, dtype=np.float32)
    Bmat = np.asarray(Bmat, dtype=np.float32)
    W_ro = np.asarray(W_ro, dtype=np.float32)

    jt = np.ascontiguousarray((DT / DECAY * J).T)              # [512, 512]
    ja = _to_bf16(jt)
    jb = _to_bf16(jt - np.asarray(ja, dtype=np.float32))
    bmr = np.ascontiguousarray((DT * Bmat[:, 0]).reshape(1, HIDDEN))  # [1, 512]
    wrt = _to_bf16(W_ro[0].reshape(CB, 128).T)                 # [128, 4]
    return [
        {
            "JA": ja,
            "JB": jb,
            "bmr": bmr,
            "wrt": wrt,
            "velT": np.ascontiguousarray(vel[c * B_LOC : (c + 1) * B_LOC].T),
        }
        for c in range(N_CORES)
    ]


def kernel(vel, J, Bmat, W_ro, _trace=False, **build_kw):
    from concourse.bass_utils import run_bass_kernel_spmd

    nc = _get_nc(**build_kw)
    in_maps = make_in_maps(vel, J, Bmat, W_ro)
    res = run_bass_kernel_spmd(
        nc, in_maps, list(range(N_CORES)), trace=_trace
    )
    out = np.concatenate([r["out"] for r in res.results], axis=0)
    out = out[:, :, None].astype(np.float32)
    if _trace:
        kernel.last_results = res
    return out


kernel.last_results = None


# revision 9
# speedup vs baseline: 2.6888x; 1.4378x over previous
"""CTRNN forward kernel for Trainium2 (8 NeuronCores, data-parallel over batch).

Reference computation (per step t, dt=0.02):
    h = h*(1-dt) + dt*(tanh(h) @ J.T + v_t @ Bmat.T)
    out_t = tanh(h) @ W_ro.T

Device mapping (per core, B_LOC=16 of the 128 batch rows):
  - Layout: [hidden-on-partitions x batch-on-free] ("hT"); hidden 512 =
    4 chunks of 128 partitions; chunk q row p = hidden index 128*q+p.
  - PSUM holds h (transposed) in 2 half banks [128, 32] (bank H holds
    chunks 2H, 2H+1 at cols 0:16 / 16:32), never evicted.
  - Split-precision bf16 J matmuls (the kernel is weight-load bound:
    fp32 LDWEIGHTS is 2-pass, f32r is 4x slower): J' = (dt/0.98)*J.T
    split as J' = A + B, A=bf16(J'), B=bf16(J'-A); y = tanh(h) rounded
    to bf16. Measured end-to-end rel err vs fp32 reference ~4e-3 (gate
    2e-2). Readout stays fp32-exact: z[p,b] = sum_c y32*W_ro on DVE
    (per-partition-scalar stt), one fp32 ones-contraction matmul per
    step into psum_ro.
  - Per-step PE stream (32 bf16 J matmuls + 1 small fp32 matmul), phase
    ordered so step t's half-B tail (stt/tanh/cast) hides under step
    t+1's phase-1 matmuls:
      phase1: psum[q] += A_qc@ya_c + B_qc@ya_c   for c in {0,1}, all q
      phase2: same for c in {2,3}, q in order 0,1,2,3
      after q1c3: stt_A, tanh_A -> y32_A, cast -> ya[0:2]; z stt c=0,1
      after q3c3: stt_B, tanh_B -> y32_B, cast -> ya[2:4]; z stt c=2,3
      ones-matmul: psum_ro[:, t] = z.T @ ones  (fp32)
  - bv outer products dt*Bmat x v_t precomputed per LBV-step block
    (PE outer product fp32 + DVE copies).
"""

import math
import os
import sys

import numpy as np

sys.path.insert(0, "/opt/trn_rl_repo")

DT = 0.02
DECAY = 1.0 - DT          # 0.98
HIDDEN = 512
BATCH = 128
T_FULL = 1024
N_CORES = 8
B_LOC = BATCH // N_CORES  # 16
CB = HIDDEN // 128        # 4 hidden chunks


def build_nc(T=T_FULL, lbv=128, ro_bank=512):
    import concourse.bass as bass
    import concourse.tile as tile
    from concourse import bacc, mybir

    f32 = mybir.dt.float32
    bf16 = mybir.dt.bfloat16
    nc = bacc.Bacc()

    ja_h = nc.declare_dram_parameter("JA", [HIDDEN, HIDDEN], bf16, isOutput=False)
    jb_h = nc.declare_dram_parameter("JB", [HIDDEN, HIDDEN], bf16, isOutput=False)
    bmr_h = nc.declare_dram_parameter("bmr", [1, HIDDEN], f32, isOutput=False)
    wrt_h = nc.declare_dram_parameter("wrt", [128, CB], f32, isOutput=False)
    velt_h = nc.declare_dram_parameter("velT", [T, B_LOC], f32, isOutput=False)
    out_h = nc.declare_dram_parameter("out", [B_LOC, T], f32, isOutput=True)

    n_ro = (T + ro_bank - 1) // ro_bank

    with tile.TileContext(nc) as tc:
        with (
            tc.tile_pool(name="singles", bufs=1) as singles,
            tc.tile_pool(name="yp", bufs=2) as yp,
            tc.tile_pool(name="y32p", bufs=4) as y32p,
            tc.tile_pool(name="zp", bufs=2) as zp,
            tc.tile_pool(name="velp", bufs=2) as velp,
            tc.tile_pool(name="bvpp", bufs=2) as bvpp,
            tc.tile_pool(name="psum", bufs=1, space="PSUM") as pp,
        ):
            # ---- weights staging ----
            jta = singles.tile([128, CB, HIDDEN], bf16, tag="jta")
            nc.sync.dma_start(out=jta, in_=ja_h.rearrange("(c p) i -> p c i", p=128))
            jtb = singles.tile([128, CB, HIDDEN], bf16, tag="jtb")
            nc.sync.dma_start(out=jtb, in_=jb_h.rearrange("(c p) i -> p c i", p=128))
            bmr = singles.tile([1, HIDDEN], f32, tag="bmr")  # dt*Bmat as a row
            nc.sync.dma_start(out=bmr, in_=bmr_h[:, :])
            wrt = singles.tile([128, CB], f32, tag="wrt")
            nc.sync.dma_start(out=wrt, in_=wrt_h[:, :])

            # zero lhsT/rhs used to clear+claim the h PSUM banks; ones for
            # the readout partition-contraction matmul
            zrow = singles.tile([1, 128], f32, tag="zrow")
            nc.vector.memset(zrow, 0.0)
            ones = singles.tile([128, 1], f32, tag="ones")
            nc.vector.memset(ones, 1.0)

            psum_z = [
                pp.tile([128, 2 * B_LOC], f32, tag=f"z{h}", name=f"psum_z{h}")
                for h in range(2)
            ]
            psum_ro = [
                pp.tile([B_LOC, ro_bank], f32, tag=f"ro{i}", name=f"psum_ro{i}")
                for i in range(n_ro)
            ]
            # junk PSUM target for "absorber" matmuls: each absorber consumes a
            # single fresh semaphore tick (DMA completion etc.) so that real
            # matmuls never need more than ONE sync wait (the self-loading
            # matmul's LDWEIGHTS slice has a single wait slot).
            pjunk = pp.tile([1, 8], f32, tag="junk", name="psum_junk")

            def absorb(src_1el):
                nc.tensor.matmul(
                    out=pjunk[0:1, 0:1],
                    lhsT=src_1el,
                    rhs=src_1el,
                    start=True,
                    stop=True,
                    skip_group_check=True,
                )

            for h in range(2):
                nc.tensor.matmul(
                    out=psum_z[h],
                    lhsT=zrow[0:1, 0:128],
                    rhs=zrow[0:1, 0 : 2 * B_LOC],
                    start=True,
                    stop=True,
                    skip_group_check=True,
                )

            # soak up the weight-staging DMA completions one at a time
            absorb(jta[0:1, 0, 0:1])
            absorb(jtb[0:1, 0, 0:1])
            absorb(wrt[0:1, 0:1])
            absorb(bmr[0:1, 0:1])

            ya_prev = yp.tile([128, CB, B_LOC], bf16, tag="ya")
            nc.vector.memset(ya_prev.rearrange("p c b -> p (c b)"), 0.0)

            def jmm(q, c, rhs):
                col = B_LOC * (q % 2)
                for jt_w in (jta, jtb):
                    nc.tensor.matmul(
                        out=psum_z[q // 2][:, col : col + B_LOC],
                        lhsT=jt_w[:, c, 128 * q : 128 * (q + 1)],
                        rhs=rhs,
                        start=False,
                        stop=False,
                        skip_group_check=True,
                    )

            velb = None
            bvp = None
            bvp_prev = None
            for t in range(T):
                j = t % lbv
                if j == 0:
                    # stage next LBV steps of dt*Bmat x v outer products, built
                    # on the PE: bvp[p, t, c, b] = bmr[128c+p] * v[t, b]
                    velb = velp.tile([1, lbv * B_LOC], f32, tag="velB")
                    nc.sync.dma_start(
                        out=velb,
                        in_=velt_h[t : t + lbv, :]
                        .rearrange("t b -> (t b)")
                        .unsqueeze(0),
                    )
                    if bvp_prev is not None:
                        # observe the previous block's last bvp copy (DVE tick)
                        # so the matmuls below only wait on the velb DMA
                        absorb(bvp_prev[0:1, lbv - 1, CB - 1, 0:1])
                    bvp_prev = bvp
                    bvp = bvpp.tile([128, lbv, CB, B_LOC], f32, tag="bvp")
                    for c in range(CB):
                        for q0 in range(0, lbv * B_LOC, 512):
                            qn = min(512, lbv * B_LOC - q0)
                            t0, nt = q0 // B_LOC, qn // B_LOC
                            pbv = pp.tile(
                                [128, 512], f32, tag="pbv", bufs=1, name="psum_bv"
                            )
                            nc.tensor.matmul(
                                out=pbv[:, 0:qn],
                                lhsT=bmr[0:1, 128 * c : 128 * (c + 1)],
                                rhs=velb[0:1, q0 : q0 + qn],
                                start=True,
                                stop=True,
                                skip_group_check=True,
                            )
                            nc.vector.tensor_copy(
                                bvp[:, t0 : t0 + nt, c, :],
                                pbv[:, 0:qn].rearrange("p (t b) -> p t b", b=B_LOC),
                            )

                ya_new = yp.tile([128, CB, B_LOC], bf16, tag="ya")
                z = zp.tile([128, B_LOC], f32, tag="z")
                rb, rc = t // ro_bank, t % ro_bank

                # phase 1: contraction chunks 0,1 for all quarters (only needs
                # ya chunks 0,1 of step t-1, which finish early)
                for q in range(CB):
                    for c in (0, 1):
                        jmm(q, c, ya_prev[:, c, :])

                # phase 2 + per-half tail
                for h in range(2):
                    for q in (2 * h, 2 * h + 1):
                        for c in (2, 3):
                            jmm(q, c, ya_prev[:, c, :])
                    # psum_h = 0.98*psum_h + dt*Bmat x v_t  (chunks 2h, 2h+1)
                    nc.vector.scalar_tensor_tensor(
                        out=psum_z[h],
                        in0=psum_z[h],
                        scalar=float(DECAY),
                        in1=bvp[:, j, 2 * h : 2 * h + 2, :].rearrange(
                            "p c b -> p (c b)"
                        ),
                        op0=mybir.AluOpType.mult,
                        op1=mybir.AluOpType.add,
                    )
                    # y32 = tanh(h); ya = bf16(y32)
                    y32 = y32p.tile([128, 2, B_LOC], f32, tag="y32")
                    nc.scalar.activation(
                        out=y32.rearrange("p c b -> p (c b)"),
                        in_=psum_z[h],
                        func=mybir.ActivationFunctionType.Tanh,
                    )
                    nc.vector.tensor_copy(
                        ya_new[:, 2 * h : 2 * h + 2, :].rearrange("p c b -> p (c b)"),
                        y32.rearrange("p c b -> p (c b)"),
                    )
                    # readout accumulator z[p,b] (+)= y32_c * W_ro[p,c]
                    for i in range(2):
                        c = 2 * h + i
                        if c == 0:
                            nc.vector.tensor_scalar(
                                out=z,
                                in0=y32[:, 0, :],
                                scalar1=wrt[:, 0:1],
                                scalar2=None,
                                op0=mybir.AluOpType.mult,
                            )
                        else:
                            nc.vector.scalar_tensor_tensor(
                                out=z,
                                in0=y32[:, i, :],
                                scalar=wrt[:, c : c + 1],
                                in1=z,
                                op0=mybir.AluOpType.mult,
                                op1=mybir.AluOpType.add,
                            )

                # psum_ro[:, t] = z.T @ ones  (fp32, exact readout)
                nc.tensor.matmul(
                    out=psum_ro[rb][0:B_LOC, rc : rc + 1],
                    lhsT=z,
                    rhs=ones,
                    start=True,
                    stop=True,
                    skip_group_check=True,
                )

                if rc == ro_bank - 1 or t == T - 1:
                    out_sb = velp.tile([B_LOC, ro_bank], f32, tag="osb", name="out_sb")
                    nc.vector.tensor_copy(out_sb[:, 0 : rc + 1], psum_ro[rb][:, 0 : rc + 1])
                    nc.sync.dma_start(
                        out=out_h[:, rb * ro_bank : rb * ro_bank + rc + 1],
                        in_=out_sb[:, 0 : rc + 1],
                    )

                ya_prev = ya_new

    nc.compile()
    return nc


_NC_CACHE = {}


def _get_nc(**kw):
    key = tuple(sorted(kw.items()))
    if key not in _NC_CACHE:
        _NC_CACHE[key] = build_nc(**kw)
    return _NC_CACHE[key]


def _to_bf16(x):
    import ml_dtypes

    return np.asarray(x, dtype=np.float32).astype(ml_dtypes.bfloat16)


def make_in_maps(vel, J, Bmat, W_ro):
    vel = np.asarray(vel, dtype=np.float32)[:, :, 0]          # [B, T]
    J = np.asarray(J, dtype=np.float32)
    Bmat = np.asarray(Bmat, dtype=np.float32)
    W_ro = np.asarray(W_ro, dtype=np.float32)

    jt = np.ascontiguousarray((DT / DECAY * J).T)              # [512, 512]
    ja = _to_bf16(jt)
    jb = _to_bf16(jt - np.asarray(ja, dtype=np.float32))
    bmr = np.ascontiguousarray((DT * Bmat[:, 0]).reshape(1, HIDDEN))  # [1, 512]
    wrt = np.ascontiguousarray(W_ro[0].reshape(CB, 128).T)     # [128, 4] fp32
    return [
        {
            "JA": ja,
            "JB": jb,
            "bmr": bmr,
            "wrt": wrt,
            "velT": np.ascontiguousarray(vel[c * B_LOC : (c + 1) * B_LOC].T),
        }
        for c in range(N_CORES)
    ]


def kernel(vel, J, Bmat, W_ro, _trace=False, **build_kw):
    from concourse.bass_utils import run_bass_kernel_spmd

    nc = _get_nc(**build_kw)
    in_maps = make_in_maps(vel, J, Bmat, W_ro)
    res = run_bass_kernel_spmd(
        nc, in_maps, list(range(N_CORES)), trace=_trace
    )
    out = np.concatenate([r["out"] for r in res.results], axis=0)
    out = out[:, :, None].astype(np.float32)
    if _trace:
        kernel.last_results = res
    return out


kernel.last_results = None
